# revision 5
# baseline (speedup 1.0000x reference)
"""Trainium2 Bass kernel for nn_DispersiveLoss (B=2048, D=16*768=12288, 8 cores).

Strategy (circulant block decomposition, uniform SPMD):
  x (2048, 12288) -> 16 row-blocks of 128. Core c "owns" m-blocks {2c, 2c+1}
  and computes two Gram strips G[m, m..m+8 (mod 16)] (width 9 blocks = 1152)
  in bf16 with D on partitions (96 k-chunks of 128, PSUM fp32 accumulation).
  Every unordered block pair lands exactly once (circular distance 1..7),
  diagonal blocks are masked to the upper triangle, distance-8 blocks are
  computed twice and weighted 0.5 -- all via one shared mask + ACT scales,
  so the program is identical on all 8 cores (pure SPMD).

  Launch A computes row norms sq_i = ||x_i||^2 for each core's own 256 rows
  (natural layout, DVE/ACT split). The host gathers/redistributes sq (pure
  data movement), then launch B consumes the Gram strips:
    u = d2 - 2D = -2*(g - (sq_n - 2D)/2) + sq_m
  where the per-column term is folded into PSUM by a K=1 ones-matmul and the
  per-partition term rides the ACT bias. ACT produces exp(-u/(D*tau)), u, u^2
  with per-instruction accum_out reductions; DVE handles the triangle-masked
  diagonal blocks. Host combines per-core (128,4) partial sums in float64.
"""

import os

import numpy as np
import ml_dtypes

import concourse.bass as bass
import concourse.mybir as mybir
import concourse.tile as tile
from concourse import bacc
from concourse.bass_utils import run_bass_kernel_spmd

NC_N = 8
B, D = 2048, 12288
BLK = 128
UNION = 1280  # 10 blocks per core in SBUF
STRIPW = 1152  # 9-block strip width
KCH = 96  # k-chunks of 128
KB = 4  # k-chunks per DMA batch
TAU = 0.5
CC = float(2 * D)  # centering constant (E[d2] for N(0,1) rows)
SS = 1.0 / (D * TAU)  # exponent scale
F32 = mybir.dt.float32
BF16 = mybir.dt.bfloat16
LN_HALF = float(np.log(0.5))
INV_SQRT2 = float(1.0 / np.sqrt(2.0))

# effective pair count: 16 * (tri 8128 + 7 full blocks + half block)
N_PAIRS = B * (B - 1) // 2

KERNEL_EXEC_NS = []  # filled when KERNEL_TRACE is set (test harness only)

_cache = {}


def _trace_enabled():
    return bool(os.environ.get("KERNEL_TRACE"))


def _build_sq_kernel():
    """Launch A: per core, sq for its own 256 rows from natural-layout bf16."""
    nc = bacc.Bacc("TRN2", target_bir_lowering=False, debug=False, num_devices=NC_N)
    xn = nc.dram_tensor("xn", [2, BLK, D], BF16, kind="ExternalInput")
    sq_out = nc.dram_tensor("sq_out", [2, BLK], F32, kind="ExternalOutput")
    CH = 3072
    NCH = D // CH  # 4

    with tile.TileContext(nc) as tc:
        with (
            tc.tile_pool(name="p", bufs=3) as p,
            tc.tile_pool(name="a", bufs=1) as a,
        ):
            acc = a.tile([BLK, 2 * NCH], F32)
            for b in range(2):
                for j in range(NCH):
                    t = p.tile([BLK, CH], BF16, tag="xin")
                    nc.sync.dma_start(t[:], xn[b, :, j * CH : (j + 1) * CH])
                    col = acc[:, b * NCH + j : b * NCH + j + 1]
                    scr = p.tile([BLK, CH], F32, tag="scr")
                    if b == 0:
                        nc.vector.scalar_tensor_tensor(
                            out=scr[:],
                            in0=t[:],
                            scalar=1.0,
                            in1=t[:],
                            op0=mybir.AluOpType.mult,
                            op1=mybir.AluOpType.mult,
                            accum_out=col,
                        )
                    else:
                        nc.scalar.activation(
                            scr[:],
                            t[:],
                            mybir.ActivationFunctionType.Square,
                            accum_out=col,
                        )
            r = a.tile([BLK, 2], F32)
            nc.vector.tensor_reduce(
                r[:, 0:1], acc[:, 0:NCH], mybir.AxisListType.X, mybir.AluOpType.add
            )
            nc.vector.tensor_reduce(
                r[:, 1:2], acc[:, NCH : 2 * NCH], mybir.AxisListType.X, mybir.AluOpType.add
            )
            for b in range(2):
                nc.sync.dma_start(
                    sq_out[b].rearrange("(p o) -> p o", o=1), r[:, b : b + 1]
                )
    nc.compile()
    return nc


def _build_main_kernel():
    """Launch B: Gram strips + statistics."""
    nc = bacc.Bacc("TRN2", target_bir_lowering=False, debug=False, num_devices=NC_N)
    xT = nc.dram_tensor("xT", [BLK, KCH, UNION], BF16, kind="ExternalInput")
    sqw = nc.dram_tensor("sqw", [UNION], F32, kind="ExternalInput")
    tri = nc.dram_tensor("tri", [BLK, BLK], F32, kind="ExternalInput")
    out_stats = nc.dram_tensor("out_stats", [BLK, 4], F32, kind="ExternalOutput")

    MULT = mybir.AluOpType.mult
    ADD = mybir.AluOpType.add
    EXP = mybir.ActivationFunctionType.Exp
    SQUARE = mybir.ActivationFunctionType.Square
    IDENT = mybir.ActivationFunctionType.Identity
    SQRT = mybir.ActivationFunctionType.Sqrt

    # acc columns: 0:6 E [mid1, mid2, half1, half2, tri1, tri2]
    #              6:12 S1, 12:18 S2, 18 feat
    with tile.TileContext(nc) as tc:
        with (
            tc.tile_pool(name="slab", bufs=3) as slab_pool,
            tc.tile_pool(name="psp", bufs=1, space="PSUM") as psp,
            tc.tile_pool(name="post", bufs=2) as post,
            tc.tile_pool(name="accp", bufs=1) as accp,
        ):
            ps0 = psp.tile([BLK, STRIPW], F32, tag="ps0")
            ps1 = psp.tile([BLK, STRIPW], F32, tag="ps1")
            ps = [ps0, ps1]
            # strip s: m-block at union col 128*s, window = union cols 128*s..128*s+1152
            segs = [(0, 512), (512, 1024), (1024, 1152)]

            for kb in range(KCH // KB):
                st = slab_pool.tile([BLK, KB, UNION], BF16, tag="slab")
                nc.sync.dma_start(st[:], xT[:, kb * KB : (kb + 1) * KB, :])
                for ii in range(KB):
                    k = kb * KB + ii
                    for s in range(2):
                        off = 128 * s
                        lhs = st[:, ii, off : off + 128]
                        for c0, c1 in segs:
                            nc.tensor.matmul(
                                ps[s][:, c0:c1],
                                lhs,
                                st[:, ii, off + c0 : off + c1],
                                start=(k == 0),
                                stop=False,
                            )

            # ---- post processing ----
            acc = accp.tile([BLK, 19], F32)
            ones = post.tile([1, BLK], F32, tag="ones")
            nc.gpsimd.memset(ones[:], 1.0)
            sqrow = post.tile([1, UNION], F32, tag="sqrow")
            nc.sync.dma_start(sqrow[:], sqw[:].rearrange("(a b) -> a b", a=1))
            vrow = post.tile([1, UNION], F32, tag="vrow")
            # v = (sq - CC) * (-0.5)
            nc.vector.tensor_scalar(
                out=vrow[:],
                in0=sqrow[:],
                scalar1=-CC,
                scalar2=-0.5,
                op0=ADD,
                op1=MULT,
            )
            # fold per-column term into PSUM: ps += ones^T @ v  (K=1 matmul)
            for s in range(2):
                off = 128 * s
                for c0, c1 in segs:
                    nc.tensor.matmul(
                        ps[s][:, c0:c1],
                        ones[:],
                        vrow[:, off + c0 : off + c1],
                        start=False,
                        stop=(c0, c1) == segs[-1],
                    )

            tri_t = post.tile([BLK, BLK], F32, tag="tri")
            nc.sync.dma_start(tri_t[:], tri[:])
            lnhalf = post.tile([BLK, 1], F32, tag="lnhalf")
            nc.gpsimd.memset(lnhalf[:], LN_HALF)

            for s in range(2):
                off = 128 * s
                bm = post.tile([BLK, 1], F32, tag=f"bias{s}")
                nc.sync.dma_start(
                    bm[:], sqw[off : off + 128].rearrange("(p o) -> p o", o=1)
                )
                u = post.tile([BLK, STRIPW], F32, tag=f"u{s}")
                # u = -2 * ps + sq_m   (= d2 - CC)
                nc.scalar.activation(u[:], ps[s][:], IDENT, bias=bm[:], scale=-2.0)

                # mid region (full-weight blocks): cols 128:1024
                um = u[:, 128:1024]
                scr = post.tile([BLK, 896], F32, tag="scr")
                nc.scalar.activation(scr[:], um, EXP, scale=-SS, accum_out=acc[:, s : s + 1])
                scr = post.tile([BLK, 896], F32, tag="scr")
                nc.scalar.activation(scr[:], um, SQUARE, accum_out=acc[:, 12 + s : 13 + s])
                scr = post.tile([BLK, 896], F32, tag="scr")
                nc.scalar.activation(scr[:], um, IDENT, accum_out=acc[:, 6 + s : 7 + s])

                # half-weight region (distance-8 block, computed twice fleet-wide):
                # cols 1024:1152; weight 0.5 folded into ACT scale/bias
                uh = u[:, 1024:1152]
                scr2 = post.tile([BLK, BLK], F32, tag="scr2")
                nc.scalar.activation(
                    scr2[:], uh, EXP, bias=lnhalf[:], scale=-SS,
                    accum_out=acc[:, 2 + s : 3 + s],
                )
                scr2 = post.tile([BLK, BLK], F32, tag="scr2")
                nc.scalar.activation(
                    scr2[:], uh, SQUARE, scale=INV_SQRT2,
                    accum_out=acc[:, 14 + s : 15 + s],
                )
                scr2 = post.tile([BLK, BLK], F32, tag="scr2")
                nc.scalar.activation(
                    scr2[:], uh, IDENT, scale=0.5, accum_out=acc[:, 8 + s : 9 + s]
                )

                # diagonal block (upper-triangle mask): cols 0:128
                ud = u[:, 0:128]
                et = post.tile([BLK, BLK], F32, tag=f"et{s}")
                nc.scalar.activation(et[:], ud, EXP, scale=-SS)
                me = post.tile([BLK, BLK], F32, tag="me")
                nc.vector.scalar_tensor_tensor(
                    out=me[:], in0=et[:], scalar=1.0, in1=tri_t[:],
                    op0=MULT, op1=MULT, accum_out=acc[:, 4 + s : 5 + s],
                )
                mu = post.tile([BLK, BLK], F32, tag=f"mu{s}")
                nc.vector.scalar_tensor_tensor(
                    out=mu[:], in0=ud, scalar=1.0, in1=tri_t[:],
                    op0=MULT, op1=MULT, accum_out=acc[:, 10 + s : 11 + s],
                )
                ms2 = post.tile([BLK, BLK], F32, tag="ms2")
                nc.vector.scalar_tensor_tensor(
                    out=ms2[:], in0=mu[:], scalar=1.0, in1=ud,
                    op0=MULT, op1=MULT, accum_out=acc[:, 16 + s : 17 + s],
                )

            # feat partial: sum sqrt(sq_own) over own 256 rows
            ft = post.tile([BLK, 2], F32, tag="ft")
            nc.sync.dma_start(ft[:], sqw[0:256].rearrange("(j p) -> p j", j=2))
            fscr = post.tile([BLK, 2], F32, tag="fscr")
            nc.scalar.activation(fscr[:], ft[:], SQRT, accum_out=acc[:, 18:19])

            outt = accp.tile([BLK, 4], F32)
            nc.vector.tensor_reduce(outt[:, 0:1], acc[:, 0:6], mybir.AxisListType.X, ADD)
            nc.vector.tensor_reduce(outt[:, 1:2], acc[:, 6:12], mybir.AxisListType.X, ADD)
            nc.vector.tensor_reduce(outt[:, 2:3], acc[:, 12:18], mybir.AxisListType.X, ADD)
            nc.vector.tensor_copy(outt[:, 3:4], acc[:, 18:19])
            nc.sync.dma_start(out_stats[:], outt[:])
    nc.compile()
    return nc


def _get(name, builder):
    if name not in _cache:
        _cache[name] = builder()
    return _cache[name]


def _run(nc, in_maps, tag):
    if _trace_enabled():
        try:
            import profhook

            profhook.install()
        except Exception:
            pass
        import tempfile

        res = run_bass_kernel_spmd(
            nc, in_maps, list(range(NC_N)), trace=True,
            tmpdir=tempfile.mkdtemp(prefix=f"ktrace_{tag}_"),
        )
        KERNEL_EXEC_NS.append((tag, res.exec_time_ns))
        return res.results
    return run_bass_kernel_spmd(nc, in_maps, list(range(NC_N))).results


def kernel(features):
    x = np.asarray(features).reshape(B, D)
    xbf = x.astype(ml_dtypes.bfloat16)

    # ---- launch A: row norms ----
    a_maps = [
        {"xn": np.ascontiguousarray(xbf[256 * c : 256 * c + 256]).reshape(2, BLK, D)}
        for c in range(NC_N)
    ]
    nc_a = _get("sq", _build_sq_kernel)
    a_res = _run(nc_a, a_maps, "sq")
    sq_full = np.concatenate([a_res[c]["sq_out"].reshape(256) for c in range(NC_N)])

    # ---- launch B: Gram strips + stats ----
    xT_full = np.ascontiguousarray(xbf.T)  # (D, B)
    b_maps = []
    tri = np.triu(np.ones((BLK, BLK), np.float32), k=1)
    for c in range(NC_N):
        cols = (256 * c + np.arange(UNION)) % B
        xu = xT_full[:, cols].reshape(KCH, BLK, UNION).transpose(1, 0, 2)
        b_maps.append(
            {
                "xT": np.ascontiguousarray(xu),
                "sqw": sq_full[cols].astype(np.float32),
                "tri": tri,
            }
        )
    nc_b = _get("main", _build_main_kernel)
    b_res = _run(nc_b, b_maps, "main")

    # ---- host combine (gather of partial sums only) ----
    E = S1 = S2 = FT = 0.0
    for c in range(NC_N):
        o = b_res[c]["out_stats"].astype(np.float64)
        E += o[:, 0].sum()
        S1 += o[:, 1].sum()
        S2 += o[:, 2].sum()
        FT += o[:, 3].sum()

    N = float(N_PAIRS)
    mean_u = S1 / N
    mean = (mean_u + CC) / D
    var_u = (S2 - N * mean_u * mean_u) / (N - 1.0)
    std = np.sqrt(var_u) / D
    # logsumexp(-pdn/tau) = -CC*SS + log(E); loss = -that + log(N)
    loss = CC * SS - np.log(E) + np.log(N)
    feat_norm = FT / B

    return (
        np.float32(loss),
        np.float32(feat_norm),
        np.float32(mean),
        np.float32(std),
    )


if __name__ == "__main__":
    f = np.random.default_rng(0).standard_normal((B, 16, 768), dtype=np.float32)
    print(kernel(features=f))
